# revision 7
# baseline (speedup 1.0000x reference)
"""Label-wise FFN kernel for Trainium2 (8 NeuronCores, label-sharded).

Computes out[b, l] = relu(x @ W1[l] + b1[l]) @ W2[l] + b2[l] for
B=8192, D=1024, L=64, H=256, fp32 in/out.

Sharding: L is split across the 8 cores (8 labels each); every core holds a
full replica of x. Each core runs both layers for its labels and writes its
[8, B] slice of the (transposed) output; the host concatenates and
transposes back to [B, L].

Per-core dataflow (matmul operands in bf16, accumulation in fp32 PSUM):
  layer 1: psum[h=128, b=512] += W1tile[d=128, h=128].T @ xT[d=128, b=512]
           over 8 d-tiles; ACT fuses bias-add (per-partition) + ReLU on the
           PSUM->SBUF copy, casting h to bf16.
  layer 2: psum2[8, b=512] += W2blk[h=128, 8].T @ h[h=128, b=512] over the
           16 (label, h-chunk) tiles, with W2blk block-diagonal so each
           output partition is one label's dot product; ACT adds b2.

bf16 weights enable the PE fast-weight-load path (fp32/f32r cannot use it),
so the per-[128x128]x[128x512] matmul cost drops from the ~265 ns f32r
fused-weight-load rate toward the 213 ns pure-stream roofline. bf16
rounding error (~2-3e-3 rel) is well inside the 2e-2 gate.
"""

import numpy as np
import ml_dtypes

import concourse.bacc as bacc
import concourse.mybir as mybir
import concourse.tile as tile
from concourse.bass_utils import run_bass_kernel_spmd

B, D, L, H = 8192, 1024, 64, 256
NCORES = 8
LPC = L // NCORES      # labels per core
P = 128
HC = H // P            # h-chunks per label
KT = D // P            # k-tiles over D
BCHUNK = 512
NB = B // BCHUNK       # b-chunks
NL2 = LPC * HC         # layer-2 k-tiles

BF16 = ml_dtypes.bfloat16


def build_nc(repeat=1, ps1_bufs=5, ps2_bufs=2, h_bufs=18, x_bufs=24, bgrp=1,
             ldw_skip=0, no_l2=0, no_act=0, x_resident=0, no_out_dma=0):
    bf16 = mybir.dt.bfloat16
    f32 = mybir.dt.float32
    nc = bacc.Bacc(None, target_bir_lowering=False)

    xT = nc.dram_tensor("xT", [D, B], bf16, kind="ExternalInput")
    w1t = nc.dram_tensor("w1t", [LPC, HC, P, KT, P], bf16, kind="ExternalInput")
    w2blk = nc.dram_tensor("w2blk", [NL2, P, LPC], bf16, kind="ExternalInput")
    b1t = nc.dram_tensor("b1t", [NL2, P], f32, kind="ExternalInput")
    b2c = nc.dram_tensor("b2c", [LPC, 1], f32, kind="ExternalInput")
    out = nc.dram_tensor("out", [LPC, B], f32, kind="ExternalOutput")

    relu = mybir.ActivationFunctionType.Relu
    ident = mybir.ActivationFunctionType.Identity

    with tile.TileContext(nc) as tc:
        with (
            tc.tile_pool(name="wpool", bufs=1) as wpool,
            tc.tile_pool(name="xpool", bufs=x_bufs) as xpool,
            tc.tile_pool(name="hpool", bufs=h_bufs) as hpool,
            tc.tile_pool(name="opool", bufs=4) as opool,
            tc.tile_pool(name="ps1", bufs=ps1_bufs, space="PSUM") as ps1pool,
            tc.tile_pool(name="ps2", bufs=ps2_bufs, space="PSUM") as ps2pool,
        ):
            # Resident weights/biases (loaded once, outside any repeat loop).
            w1sb = []
            for l in range(LPC):
                t = wpool.tile([P, HC, KT, P], bf16, tag=f"w1_{l}")
                for hc in range(HC):
                    nc.sync.dma_start(t[:, hc], w1t[l, hc])
                w1sb.append(t)
            w2sb = wpool.tile([P, NL2, LPC], bf16, tag="w2")
            nc.sync.dma_start(w2sb[:], w2blk.rearrange("n p j -> p n j"))
            b1sb = wpool.tile([P, NL2], f32, tag="b1")
            nc.sync.dma_start(b1sb[:], b1t.rearrange("n p -> p n"))
            b2sb = wpool.tile([LPC, 1], f32, tag="b2")
            nc.sync.dma_start(b2sb[:], b2c[:])

            xT_t = xT.rearrange("(k p) b -> p k b", p=P)

            xk_res = None
            if x_resident:
                xk_res = []
                for c in range(NB):
                    bs = c * BCHUNK
                    row = []
                    for kt in range(KT):
                        t = xpool.tile(
                            [P, BCHUNK], bf16, name="xr", tag=f"xr{c}_{kt}"
                        )
                        nc.sync.dma_start(t[:], xT_t[:, kt, bs : bs + BCHUNK])
                        row.append(t)
                    xk_res.append(row)

            def body():
                # Process b-chunks in groups of `bgrp`: inside a group the
                # same W1 tile feeds bgrp consecutive matmuls (one per
                # chunk), amortizing the stationary-weight load.
                for g in range(NB // bgrp):
                    if x_resident:
                        xk = [xk_res[g * bgrp + j] for j in range(bgrp)]
                    else:
                        xk = []   # xk[j][kt]
                        for j in range(bgrp):
                            bs = (g * bgrp + j) * BCHUNK
                            row = []
                            for kt in range(KT):
                                t = xpool.tile([P, BCHUNK], bf16, tag="xk")
                                nc.sync.dma_start(
                                    t[:], xT_t[:, kt, bs : bs + BCHUNK]
                                )
                                row.append(t)
                            xk.append(row)
                    hts = [[] for _ in range(bgrp)]
                    ps1s = [None] * bgrp
                    for l in range(LPC):
                        for hc in range(HC):
                            idx = l * HC + hc
                            for j in range(bgrp):
                                ps1s[j] = ps1pool.tile(
                                    [P, BCHUNK], f32, name="ps1", tag="ps1"
                                )
                            for kt in range(KT):
                                for j in range(bgrp):
                                    mm = nc.tensor.matmul(
                                        ps1s[j][:],
                                        w1sb[l][:, hc, kt],
                                        xk[j][kt][:],
                                        start=(kt == 0),
                                        stop=(kt == KT - 1),
                                    )
                                    if ldw_skip and j > 0:
                                        # Reuse the stationary tile loaded by
                                        # the j==0 matmul of this kt.
                                        mm.ins.ldweights = False
                            if not no_act:
                                for j in range(bgrp):
                                    ht = hpool.tile(
                                        [P, BCHUNK], bf16, tag="ht"
                                    )
                                    nc.scalar.activation(
                                        ht[:], ps1s[j][:], relu,
                                        bias=b1sb[:, idx : idx + 1],
                                    )
                                    hts[j].append(ht)
                    if no_l2 or no_act:
                        continue
                    for j in range(bgrp):
                        bs = (g * bgrp + j) * BCHUNK
                        ps2 = ps2pool.tile([LPC, BCHUNK], f32)
                        for idx, ht in enumerate(hts[j]):
                            nc.tensor.matmul(
                                ps2[:],
                                w2sb[:, idx],
                                ht[:],
                                start=(idx == 0),
                                stop=(idx == NL2 - 1),
                            )
                        ot = opool.tile([LPC, BCHUNK], f32)
                        nc.scalar.activation(
                            ot[:], ps2[:], ident, bias=b2sb[:, 0:1]
                        )
                        if not no_out_dma:
                            nc.sync.dma_start(
                                out[:, bs : bs + BCHUNK], ot[:]
                            )

            if repeat > 1:
                with tc.For_i(0, repeat, 1):
                    body()
            else:
                body()

            if no_l2 or no_act or no_out_dma:
                # Ablation variants: emit one small output write so the
                # module still has a produced ExternalOutput.
                nc.sync.dma_start(out[0:LPC, 0:NL2], b1sb[0:LPC, 0:NL2])

    nc.compile()
    return nc


def make_in_maps(x, W1, b1, W2, b2):
    """Shard + lay out the full inputs into per-core input maps."""
    x = np.asarray(x, dtype=np.float32)
    W1 = np.asarray(W1, dtype=np.float32)
    b1 = np.asarray(b1, dtype=np.float32)
    W2 = np.asarray(W2, dtype=np.float32)
    b2 = np.asarray(b2, dtype=np.float32)

    xT = np.ascontiguousarray(x.T).astype(BF16)  # [D, B], shared replica
    in_maps = []
    for core in range(NCORES):
        sl = slice(core * LPC, (core + 1) * LPC)
        w1s = W1[sl].astype(BF16)  # [LPC, D, H]
        # [LPC, D, H] -> [LPC, HC, 128(d), KT, 128(h)]
        w1tile = np.ascontiguousarray(
            w1s.reshape(LPC, KT, P, HC, P).transpose(0, 3, 2, 1, 4)
        )
        w2s = W2[sl].astype(BF16)  # [LPC, H]
        w2b = np.zeros((NL2, P, LPC), dtype=BF16)
        for l in range(LPC):
            for hc in range(HC):
                w2b[l * HC + hc, :, l] = w2s[l, hc * P : (hc + 1) * P]
        b1s = b1[sl]  # [LPC, H]
        b1tile = np.ascontiguousarray(b1s.reshape(NL2, P))
        b2s = np.ascontiguousarray(b2[sl].reshape(LPC, 1))
        in_maps.append(
            {
                "xT": xT,
                "w1t": w1tile,
                "w2blk": w2b,
                "b1t": b1tile,
                "b2c": b2s,
            }
        )
    return in_maps


def kernel(x, W1, b1, W2, b2):
    nc = build_nc()
    in_maps = make_in_maps(x, W1, b1, W2, b2)
    res = run_bass_kernel_spmd(nc, in_maps, core_ids=list(range(NCORES)))
    outs = [res.results[c]["out"] for c in range(NCORES)]  # each [LPC, B]
    full = np.concatenate(outs, axis=0)  # [L, B]
    return np.ascontiguousarray(full.T).astype(np.float32)  # [B, L]



# revision 10
# speedup vs baseline: 1.1624x; 1.1624x over previous
"""Label-wise FFN kernel for Trainium2 (8 NeuronCores, label-sharded).

Computes out[b, l] = relu(x @ W1[l] + b1[l]) @ W2[l] + b2[l] for
B=8192, D=1024, L=64, H=256, fp32 in/out.

Sharding: L is split across the 8 cores (8 labels each); every core holds a
full replica of x. Each core runs both layers for its labels and writes its
[8, B] slice of the (transposed) output; the host concatenates and
transposes back to [B, L].

Per-core dataflow (matmul operands in bf16, accumulation in fp32 PSUM):
  layer 1: psum[h=128, b=512] += W1tile[d=128, h=128].T @ xT[d=128, b=512]
           over 8 d-tiles; ACT fuses bias-add (per-partition) + ReLU on the
           PSUM->SBUF copy, casting h to bf16.
  layer 2: psum2[8, b=512] += W2blk[h=128, 8].T @ h[h=128, b=512] over the
           16 (label, h-chunk) tiles, with W2blk block-diagonal so each
           output partition is one label's dot product; ACT adds b2.

bf16 weights enable the PE fast-weight-load path (fp32/f32r cannot use it),
so the per-[128x128]x[128x512] matmul cost drops from the ~265 ns f32r
fused-weight-load rate toward the 213 ns pure-stream roofline. bf16
rounding error (~2-3e-3 rel) is well inside the 2e-2 gate.
"""

import numpy as np
import ml_dtypes

import concourse.bacc as bacc
import concourse.mybir as mybir
import concourse.tile as tile
from concourse.bass_utils import run_bass_kernel_spmd

B, D, L, H = 8192, 1024, 64, 256
NCORES = 8
LPC = L // NCORES      # labels per core
P = 128
HC = H // P            # h-chunks per label
KT = D // P            # k-tiles over D
BCHUNK = 512
NB = B // BCHUNK       # b-chunks
NL2 = LPC * HC         # layer-2 k-tiles

BF16 = ml_dtypes.bfloat16


def build_nc(repeat=1, ps1_bufs=5, ps2_bufs=2, h_bufs=18, x_bufs=24, bgrp=1,
             ldw_skip=0, no_l2=0, no_act=0, x_resident=0, no_out_dma=0,
             x_batch=0):
    bf16 = mybir.dt.bfloat16
    f32 = mybir.dt.float32
    nc = bacc.Bacc(None, target_bir_lowering=False)

    xT = nc.dram_tensor("xT", [D, B], bf16, kind="ExternalInput")
    w1t = nc.dram_tensor("w1t", [LPC, HC, P, KT, P], bf16, kind="ExternalInput")
    w2blk = nc.dram_tensor("w2blk", [NL2, P, LPC], bf16, kind="ExternalInput")
    b1t = nc.dram_tensor("b1t", [NL2, P], f32, kind="ExternalInput")
    b2c = nc.dram_tensor("b2c", [LPC, 1], f32, kind="ExternalInput")
    out = nc.dram_tensor("out", [LPC, B], f32, kind="ExternalOutput")

    relu = mybir.ActivationFunctionType.Relu
    ident = mybir.ActivationFunctionType.Identity

    with tile.TileContext(nc) as tc:
        with (
            tc.tile_pool(name="wpool", bufs=1) as wpool,
            tc.tile_pool(name="xpool", bufs=x_bufs) as xpool,
            tc.tile_pool(name="hpool", bufs=h_bufs) as hpool,
            tc.tile_pool(name="opool", bufs=4) as opool,
            tc.tile_pool(name="ps1", bufs=ps1_bufs, space="PSUM") as ps1pool,
            tc.tile_pool(name="ps2", bufs=ps2_bufs, space="PSUM") as ps2pool,
        ):
            # Resident weights/biases (loaded once, outside any repeat loop).
            w1sb = []
            for l in range(LPC):
                t = wpool.tile([P, HC, KT, P], bf16, tag=f"w1_{l}")
                for hc in range(HC):
                    nc.sync.dma_start(t[:, hc], w1t[l, hc])
                w1sb.append(t)
            w2sb = wpool.tile([P, NL2, LPC], bf16, tag="w2")
            nc.sync.dma_start(w2sb[:], w2blk.rearrange("n p j -> p n j"))
            b1sb = wpool.tile([P, NL2], f32, tag="b1")
            nc.sync.dma_start(b1sb[:], b1t.rearrange("n p -> p n"))
            b2sb = wpool.tile([LPC, 1], f32, tag="b2")
            nc.sync.dma_start(b2sb[:], b2c[:])

            xT_t = xT.rearrange("(k p) b -> p k b", p=P)

            xk_res = None
            if x_resident:
                xk_res = []
                for c in range(NB):
                    bs = c * BCHUNK
                    row = []
                    for kt in range(KT):
                        t = xpool.tile(
                            [P, BCHUNK], bf16, name="xr", tag=f"xr{c}_{kt}"
                        )
                        nc.sync.dma_start(t[:], xT_t[:, kt, bs : bs + BCHUNK])
                        row.append(t)
                    xk_res.append(row)

            def body():
                # Process b-chunks in groups of `bgrp`: inside a group the
                # same W1 tile feeds bgrp consecutive matmuls (one per
                # chunk), amortizing the stationary-weight load.
                for g in range(NB // bgrp):
                    if x_resident:
                        xk = [xk_res[g * bgrp + j] for j in range(bgrp)]
                    elif x_batch:
                        # One DMA per b-chunk loads all KT k-tiles at once.
                        xk = []
                        for j in range(bgrp):
                            bs = (g * bgrp + j) * BCHUNK
                            t = xpool.tile(
                                [P, KT, BCHUNK], bf16, name="xg", tag="xg"
                            )
                            nc.sync.dma_start(
                                t[:], xT_t[:, :, bs : bs + BCHUNK]
                            )
                            xk.append([t[:, kt] for kt in range(KT)])
                    else:
                        xk = []   # xk[j][kt]
                        for j in range(bgrp):
                            bs = (g * bgrp + j) * BCHUNK
                            row = []
                            for kt in range(KT):
                                t = xpool.tile([P, BCHUNK], bf16, tag="xk")
                                nc.sync.dma_start(
                                    t[:], xT_t[:, kt, bs : bs + BCHUNK]
                                )
                                row.append(t)
                            xk.append(row)
                    hts = [[] for _ in range(bgrp)]
                    ps1s = [None] * bgrp
                    for l in range(LPC):
                        for hc in range(HC):
                            idx = l * HC + hc
                            for j in range(bgrp):
                                ps1s[j] = ps1pool.tile(
                                    [P, BCHUNK], f32, name="ps1", tag="ps1"
                                )
                            for kt in range(KT):
                                for j in range(bgrp):
                                    rhs = (
                                        xk[j][kt]
                                        if x_batch and not x_resident
                                        else xk[j][kt][:]
                                    )
                                    mm = nc.tensor.matmul(
                                        ps1s[j][:],
                                        w1sb[l][:, hc, kt],
                                        rhs,
                                        start=(kt == 0),
                                        stop=(kt == KT - 1),
                                    )
                                    if ldw_skip and j > 0:
                                        # Reuse the stationary tile loaded by
                                        # the j==0 matmul of this kt.
                                        mm.ins.ldweights = False
                            if not no_act:
                                for j in range(bgrp):
                                    ht = hpool.tile(
                                        [P, BCHUNK], bf16, tag="ht"
                                    )
                                    nc.scalar.activation(
                                        ht[:], ps1s[j][:], relu,
                                        bias=b1sb[:, idx : idx + 1],
                                    )
                                    hts[j].append(ht)
                    if no_l2 or no_act:
                        continue
                    for j in range(bgrp):
                        bs = (g * bgrp + j) * BCHUNK
                        ps2 = ps2pool.tile([LPC, BCHUNK], f32)
                        for idx, ht in enumerate(hts[j]):
                            nc.tensor.matmul(
                                ps2[:],
                                w2sb[:, idx],
                                ht[:],
                                start=(idx == 0),
                                stop=(idx == NL2 - 1),
                            )
                        ot = opool.tile([LPC, BCHUNK], f32)
                        nc.scalar.activation(
                            ot[:], ps2[:], ident, bias=b2sb[:, 0:1]
                        )
                        if not no_out_dma:
                            nc.sync.dma_start(
                                out[:, bs : bs + BCHUNK], ot[:]
                            )

            if repeat > 1:
                with tc.For_i(0, repeat, 1):
                    body()
            else:
                body()

            if no_l2 or no_act or no_out_dma:
                # Ablation variants: emit one small output write so the
                # module still has a produced ExternalOutput.
                nc.sync.dma_start(out[0:LPC, 0:NL2], b1sb[0:LPC, 0:NL2])

    nc.compile()
    return nc


def make_in_maps(x, W1, b1, W2, b2):
    """Shard + lay out the full inputs into per-core input maps."""
    x = np.asarray(x, dtype=np.float32)
    W1 = np.asarray(W1, dtype=np.float32)
    b1 = np.asarray(b1, dtype=np.float32)
    W2 = np.asarray(W2, dtype=np.float32)
    b2 = np.asarray(b2, dtype=np.float32)

    xT = np.ascontiguousarray(x.T).astype(BF16)  # [D, B], shared replica
    in_maps = []
    for core in range(NCORES):
        sl = slice(core * LPC, (core + 1) * LPC)
        w1s = W1[sl].astype(BF16)  # [LPC, D, H]
        # [LPC, D, H] -> [LPC, HC, 128(d), KT, 128(h)]
        w1tile = np.ascontiguousarray(
            w1s.reshape(LPC, KT, P, HC, P).transpose(0, 3, 2, 1, 4)
        )
        w2s = W2[sl].astype(BF16)  # [LPC, H]
        w2b = np.zeros((NL2, P, LPC), dtype=BF16)
        for l in range(LPC):
            for hc in range(HC):
                w2b[l * HC + hc, :, l] = w2s[l, hc * P : (hc + 1) * P]
        b1s = b1[sl]  # [LPC, H]
        b1tile = np.ascontiguousarray(b1s.reshape(NL2, P))
        b2s = np.ascontiguousarray(b2[sl].reshape(LPC, 1))
        in_maps.append(
            {
                "xT": xT,
                "w1t": w1tile,
                "w2blk": w2b,
                "b1t": b1tile,
                "b2c": b2s,
            }
        )
    return in_maps


def kernel(x, W1, b1, W2, b2):
    nc = build_nc()
    in_maps = make_in_maps(x, W1, b1, W2, b2)
    res = run_bass_kernel_spmd(nc, in_maps, core_ids=list(range(NCORES)))
    outs = [res.results[c]["out"] for c in range(NCORES)]  # each [LPC, B]
    full = np.concatenate(outs, axis=0)  # [L, B]
    return np.ascontiguousarray(full.T).astype(np.float32)  # [B, L]

